# revision 51
# baseline (speedup 1.0000x reference)
"""Cosine-similarity kernel, v2: w staged transposed (d-major) from host.

Per core: x shard [2048,512] fp16 row-major, w staged as wt=[512,4096] fp16
(d-major). w never touches the PE for layout: its column norms are computed
in place via squares (ACT/DVE) + cross-chunk adds + GPSIMD
partition_all_reduce + rsqrt, then folded into the moving operand with DVE
multiplies. x is PE-transposed raw against the identity and 1/||x_b|| is
applied as a per-partition scale at PSUM eviction. GEMM runs as 8 column
panels of 16 [128,512] output groups, each DMA'd out immediately.
"""
import numpy as np

B, D, N = 16384, 512, 4096
NCORES = 8
BS = B // NCORES          # 2048 rows per core
MT = BS // 128            # 16 x row-tiles
KC = D // 128             # 4 k-chunks
NB = N // 512             # 8 column panels

_cached = {}


def _build():
    import concourse.bass_isa as bass_isa
    import concourse.mybir as mybir
    import concourse.tile as tile
    from concourse import bacc
    from concourse.masks import make_identity

    F32, F16 = mybir.dt.float32, mybir.dt.float16
    AOP = mybir.AluOpType

    nc = bacc.Bacc(None, target_bir_lowering=False)
    x = nc.dram_tensor("x", [BS, D], F16, kind="ExternalInput")
    w = nc.dram_tensor("weights", [D, N], F16, kind="ExternalInput")
    o = nc.dram_tensor("out", [BS, N], F32, kind="ExternalOutput")

    with tile.TileContext(nc) as tc:
        with (
            tc.tile_pool(name="const", bufs=1) as const,
            tc.tile_pool(name="stage", bufs=9) as stage,
            tc.tile_pool(name="wnp", bufs=3) as wnp,
            tc.tile_pool(name="norm", bufs=6) as norm,
            tc.tile_pool(name="outs", bufs=10) as outs,
            tc.tile_pool(name="mmps", bufs=6, space="PSUM") as mmps,
            tc.tile_pool(name="trps", bufs=2, space="PSUM") as trps,
        ):
            ident = const.tile([128, 128], F16, name="ident")
            make_identity(nc, ident[:])

            xT = const.tile([128, KC * BS], F16, name="xT")
            xT3 = xT[:].rearrange("p (k n) -> p k n", k=KC)
            rx = const.tile([128, MT], F32, name="rx")

            rr = [0]  # ACT/DVE round-robin for copies / scaled evictions

            # Warm the ACT Square/Sqrt table before inputs arrive.
            wu = norm.tile([128, 1], F16, name="wu", tag="wu")
            wss = norm.tile([128, 1], F32, name="wss", tag="wss")
            nc.scalar.activation(
                wu[:], ident[:, 0:1], mybir.ActivationFunctionType.Square,
                accum_out=wss[:])
            wsq = norm.tile([128, 1], F32, name="wsq", tag="wsq")
            nc.scalar.sqrt(wsq[:], wss[:])

            def load_x(b):
                """One DMA pulling 4 consecutive x 128-row tiles."""
                t4 = stage.tile([128, 4 * D], F16, name="t4", tag="ld",
                                bufs=5)
                dst = t4.rearrange("p (g d) -> p g d", g=4)
                srcap = x[b * 512 : (b + 1) * 512, :].rearrange(
                    "(g p) d -> p g d", p=128)
                nc.sync.dma_start(dst, srcap)
                return t4

            def load_w(c0, wc):
                """One DMA pulling wt[:, c0:c0+wc] as [128, KC, wc]."""
                t = stage.tile([128, KC * wc], F16, name="tw",
                               tag=f"lw{wc}", bufs=3 if wc == 512 else 4)
                dst = t.rearrange("p (k c) -> p k c", k=KC)
                srcap = w[:, c0 : c0 + wc].rearrange("(k p) c -> p k c",
                                                     p=128)
                nc.sync.dma_start(dst, srcap)
                return t

            def w_chain(t, wc, wn, dst0):
                """Normalize a loaded w block into wn[:, :, dst0:dst0+wc].

                t: [128, KC*wc] raw d-major block. Norm = rsqrt of the
                partition+chunk sum of squares, broadcast-multiplied in.
                """
                t3 = t.rearrange("p (k c) -> p k c", k=KC)
                sq = stage.tile([128, KC * wc], F16, name="sqw",
                                tag=f"sw{wc}", bufs=2 if wc == 512 else 4)
                sq3 = sq.rearrange("p (k c) -> p k c", k=KC)
                for k in range(KC):
                    if k % 2 == 0:
                        nc.scalar.activation(
                            sq3[:, k, :], t3[:, k, :],
                            mybir.ActivationFunctionType.Square)
                    else:
                        nc.vector.tensor_mul(sq3[:, k, :], t3[:, k, :],
                                             t3[:, k, :])
                u = norm.tile([128, wc], F16, name="u", tag=f"u{wc}",
                                bufs=2 if wc == 512 else 4)
                nc.vector.tensor_add(u[:], sq3[:, 0, :], sq3[:, 1, :])
                v = norm.tile([128, wc], F16, name="v", tag=f"v{wc}",
                                bufs=2 if wc == 512 else 4)
                nc.gpsimd.tensor_add(v[:], sq3[:, 2, :], sq3[:, 3, :])
                ssq = norm.tile([128, wc], F16, name="ssq", tag=f"s{wc}",
                                bufs=2 if wc == 512 else 4)
                nc.vector.tensor_add(ssq[:], u[:], v[:])
                ssf = norm.tile([128, wc], F32, name="ssf", tag=f"f{wc}",
                                bufs=2 if wc == 512 else 4)
                nc.gpsimd.partition_all_reduce(
                    ssf[:], ssq[:], 128, bass_isa.ReduceOp.add)
                inv = norm.tile([128, wc], F32, name="invw", tag=f"i{wc}",
                                bufs=2 if wc == 512 else 4)
                nc.vector.reciprocal(inv[:], ssf[:])
                rw = norm.tile([128, wc], F16, name="rww", tag=f"r{wc}",
                                bufs=2 if wc == 512 else 4)
                nc.scalar.sqrt(rw[:], inv[:])
                wn3 = wn.rearrange("p (k c) -> p k c", k=KC)
                for k in range(KC):
                    if k == 3:
                        nc.gpsimd.tensor_mul(
                            wn3[:, k, dst0 : dst0 + wc], t3[:, k, :], rw[:])
                    else:
                        nc.vector.tensor_mul(
                            wn3[:, k, dst0 : dst0 + wc], t3[:, k, :], rw[:])

            def x_t(t4, g, m):
                """PE-transpose raw x tile m into xT."""
                t = t4[:, g * D : (g + 1) * D]
                pt = trps.tile([128, KC * 128], F32, name="pt", tag="pt")
                for k in range(KC):
                    nc.tensor.matmul(
                        pt[:, k * 128 : (k + 1) * 128],
                        t[:, k * 128 : (k + 1) * 128], ident[:],
                        start=True, stop=True)
                dst = xT3[:, :, m * 128 : (m + 1) * 128]
                src = pt[:].rearrange("p (k c) -> p k c", k=KC)
                if rr[0] % 2 == 0:
                    nc.scalar.copy(dst, src)
                else:
                    nc.vector.tensor_copy(dst, src)
                rr[0] += 1

            def x_n(t4, g, m):
                """rx[:, m] = 1/||x tile m rows|| (needed at evictions)."""
                t = t4[:, g * D : (g + 1) * D]
                sq = stage.tile([128, D], F16, name="sqx", tag="sq",
                                bufs=6)
                ss = norm.tile([128, 1], F32, name="ss", tag="ss")
                if g % 2 == 0:
                    nc.scalar.activation(
                        sq[:], t, mybir.ActivationFunctionType.Square,
                        accum_out=ss[:])
                else:
                    # tensor_tensor_reduce faults on hardware; square+reduce
                    nc.vector.tensor_mul(sq[:], t, t)
                    nc.vector.tensor_reduce(
                        ss[:], sq[:], mybir.AxisListType.XYZW, AOP.add)
                inv = norm.tile([128, 1], F32, name="inv", tag="inv")
                nc.vector.reciprocal(inv[:], ss[:])
                nc.scalar.sqrt(rx[:, m : m + 1], inv[:])

            def gemm_group(m, nb, wn3, quarters=False):
                """One [128, 512] output tile: matmuls, scaled evict, DMA."""
                pm = mmps.tile([128, 512], F32, name="pm", tag="pm")
                if quarters:
                    for q in range(4):
                        for k in range(KC):
                            nc.tensor.matmul(
                                pm[:, q * 128 : (q + 1) * 128],
                                xT3[:, k, m * 128 : (m + 1) * 128],
                                wn3[:, k, q * 128 : (q + 1) * 128],
                                start=(k == 0), stop=(k == KC - 1))
                else:
                    for k in range(KC):
                        nc.tensor.matmul(
                            pm[:],
                            xT3[:, k, m * 128 : (m + 1) * 128],
                            wn3[:, k, :],
                            start=(k == 0), stop=(k == KC - 1))
                ot = outs.tile([128, 512], F32, name="ot", tag="ot")
                if rr[0] % 2 == 0:
                    nc.scalar.mul(ot[:], pm[:], rx[:, m : m + 1])
                else:
                    nc.vector.tensor_scalar_mul(ot[:], pm[:],
                                                rx[:, m : m + 1])
                rr[0] += 1
                nc.sync.dma_start(
                    o[m * 128 : (m + 1) * 128, nb * 512 : (nb + 1) * 512],
                    ot[:])

            # ---- emission schedule ----
            # Dummy matmuls hold the PE p-state ramp until data lands.
            for _ in range(36):
                pd = trps.tile([128, KC * 128], F32, name="pd", tag="pt")
                nc.tensor.matmul(pd[:, 0:128], ident[:], ident[:],
                                 start=True, stop=True)

            tx = [load_x(0)]
            twq = [load_w(q * 128, 128) for q in range(4)]
            tx += [load_x(b) for b in range(1, 4)]

            wn_blocks = [wnp.tile([128, KC * 512], F16, name=f"wn{i}",
                                  tag="wn") for i in range(2)]
            wn0 = wn_blocks[0]
            for g in range(4):
                x_t(tx[0], g, g)
            for q in range(4):
                w_chain(twq[q], 128, wn0, q * 128)
            for g in range(4):
                x_n(tx[0], g, g)

            # Panels. wn tiles ping-pong; block nb+1 is loaded a panel ahead
            # and its chain emitted mid-panel.
            wnext = [load_w(512, 512)]
            for nb in range(NB):
                wn_cur = wn_blocks[nb % 2]
                wn3 = wn_cur.rearrange("p (k c) -> p k c", k=KC)
                for m in range(MT):
                    if nb == 0 and m in (1, 3, 5):
                        b = (m + 1) // 2
                        for g in range(4):
                            x_t(tx[b], g, 4 * b + g)
                    if nb == 0 and m in (2, 7, 10):
                        b = {2: 1, 7: 2, 10: 3}[m]
                        for g in range(4):
                            x_n(tx[b], g, 4 * b + g)
                    if m == 5 and nb < NB - 1:
                        wn_nxt = wnp.tile([128, KC * 512], F16,
                                          name=f"wnn{nb}", tag="wn")
                        wn_blocks[(nb + 1) % 2] = wn_nxt
                        w_chain(wnext[0], 512, wn_nxt, 0)
                    gemm_group(m, nb, wn3, quarters=(nb == 0))
                if nb + 2 < NB:
                    wnext[0] = load_w((nb + 2) * 512, 512)

    nc.compile()
    return nc


def kernel(x: np.ndarray, weights: np.ndarray) -> np.ndarray:
    from concourse.bass_utils import run_bass_kernel_spmd

    if "nc" not in _cached:
        _cached["nc"] = _build()
    nc = _cached["nc"]

    x16 = np.ascontiguousarray(x, dtype=np.float16)
    wt16 = np.ascontiguousarray(np.asarray(weights, dtype=np.float16).T)
    in_maps = [
        {"x": x16[i * BS : (i + 1) * BS], "weights": wt16}
        for i in range(NCORES)
    ]
    res = run_bass_kernel_spmd(nc, in_maps, list(range(NCORES)))
    return np.concatenate([res.results[i]["out"] for i in range(NCORES)], axis=0)
